# revision 19
# baseline (speedup 1.0000x reference)
"""Trainium2 Bass kernel for nn_Encoder_81595788689580.

Attention-gated GRU encoder: per time step
    w1 = h @ attn1_W.T + attn1_b
    w2 = x_t @ attn2_W.T + attn2_b
    v  = tanh(w1 + w2) @ attn3_W.T + attn3_b
    alpha = softmax(v, axis=feature)
    wx = x_t * alpha
    GRU cell (r, z, n) -> h_new
Output: [B, T, H] hidden states.

Strategy (8 NeuronCores, data-parallel over batch; 512 rows/core run as
2 pipelined chunks of 256):
  - transposed layout: features on partitions, batch on the free dim;
    weights-stationary matmuls, biases ride inside the matmuls as
    ones-rows planted in the zero-padding (x feature 320 = 1.0, and an
    fp16 ones-row slot in the h state tile).
  - x-side contractions in fp8(e4m3): the (x0,x1) 256-row pair runs as
    one DoubleRow matmul (2x), x2 as a plain fp8 matmul carrying the
    bias row; h-side contractions stay fp16 (exact recurrent path,
    keeps PE busy enough to hold the 2.4GHz p-state, and avoids an
    h->fp8 cast on the critical path).
  - wx is stored *S (S=16) in fp8 (normal range), descaled by the ACT
    `scale` at each gate activation; the r/z h-side weights carry S,
    the n-path h-side weights carry 0.5*S (folding the sigmoid scale).
  - precision: n-gate path fp16 end-to-end; sim rel err ~2e-3.
  - chain shortening: h-only matmuls (t1h, bias rows) fill the PE
    while ACT runs; the gate ACT is split r-first/z-second so the
    n-gate chain starts after only the r blocks; the z-path products
    (zz, w1z on DVE, bzh on gpsimd) run off the critical chain.
  - softmax denominator: fp16 ones-matmul (value 1/S) so reciprocal
    yields the S-scaled normalizer directly (read as f32 by the DVE
    alpha multiply; no cast hop).
"""

import numpy as np

B, T, I, H = 4096, 24, 320, 256
NCORES = 8
BS = B // NCORES          # 512 rows per core
NCHUNK = 2
CB = BS // NCHUNK         # 256 batch columns per chunk
IP = 384                  # I padded to 3*128
KI = IP // 128            # 3 feature blocks
KH = H // 128             # 2 hidden blocks
S = 16.0                  # wx / gate-psum scale (fp8 range)

_STATE = {}


def _build(t_steps=T):
    import concourse.bass as bass
    import concourse.tile as tile
    from concourse import bacc, mybir

    f32 = mybir.dt.float32
    f16 = mybir.dt.float16
    f8 = mybir.dt.float8e4
    AF = mybir.ActivationFunctionType
    OP = mybir.AluOpType
    DR = mybir.MatmulPerfMode.DoubleRow

    nc = bacc.Bacc("TRN2", target_bir_lowering=False, debug=False,
                   num_devices=NCORES)

    x8d = nc.dram_tensor("x8", [t_steps, NCHUNK, 128, KI, CB], f8,
                         kind="ExternalInput").ap()
    x16d = nc.dram_tensor("x16", [t_steps, NCHUNK, 128, KI, CB], f16,
                          kind="ExternalInput").ap()
    h016d = nc.dram_tensor("h016", [NCHUNK, 128, KH, CB], f16,
                           kind="ExternalInput").ap()
    wu8d = nc.dram_tensor("wu8", [128, 3, IP], f8,
                          kind="ExternalInput").ap()
    wv8d = nc.dram_tensor("wv8", [128, 2, 2, IP], f8,
                          kind="ExternalInput").ap()
    wrz8d = nc.dram_tensor("wrz8", [128, 3, 512], f8,
                           kind="ExternalInput").ap()
    wp8d = nc.dram_tensor("wp8", [128, 3, 256], f8,
                          kind="ExternalInput").ap()
    wu16d = nc.dram_tensor("wu16", [128, 2, IP], f16,
                           kind="ExternalInput").ap()
    wrz16d = nc.dram_tensor("wrz16", [128, 2, 512], f16,
                            kind="ExternalInput").ap()
    wt16d = nc.dram_tensor("wt16", [128, 3, 256], f16,
                           kind="ExternalInput").ap()
    wbin16d = nc.dram_tensor("wbin16", [128, 256], f16,
                             kind="ExternalInput").ap()
    ones16d = nc.dram_tensor("ones16", [128, 128], f16,
                             kind="ExternalInput").ap()
    uz8d = nc.dram_tensor("uz8", [128, CB], f8, kind="ExternalInput").ap()
    h2c16d = nc.dram_tensor("h2c16", [128, CB], f16,
                            kind="ExternalInput").ap()
    outd = nc.dram_tensor("outT", [t_steps, NCHUNK, 128, KH, CB], f16,
                          kind="ExternalOutput").ap()

    MH_BUFS = 4
    U_BUFS = 3
    H_BUFS = 4

    def ms(m):
        return slice(m * 128, (m + 1) * 128)

    with tile.TileContext(nc) as tc:
        with tc.tile_pool(name="const", bufs=1) as cp, \
             tc.tile_pool(name="mhp", bufs=1) as mp, \
             tc.tile_pool(name="wk", bufs=1) as wp, \
             tc.tile_pool(name="ps", bufs=1, space="PSUM") as pp:

            wu8 = cp.tile([128, 3, IP], f8)
            wv8 = cp.tile([128, 2, 2, IP], f8)
            wrz8 = cp.tile([128, 3, 512], f8)
            wp8 = cp.tile([128, 3, 256], f8)
            wu16 = cp.tile([128, 2, IP], f16)
            wrz16 = cp.tile([128, 2, 512], f16)
            wt16 = cp.tile([128, 3, 256], f16)
            wbin16 = cp.tile([128, 256], f16)
            ones16 = cp.tile([128, 128], f16)
            for i, (dst, src) in enumerate([
                    (wu8, wu8d), (wv8, wv8d), (wrz8, wrz8d), (wp8, wp8d),
                    (wu16, wu16d), (wrz16, wrz16d), (wt16, wt16d),
                    (wbin16, wbin16d), (ones16, ones16d)]):
                eng = nc.sync if i % 2 == 0 else nc.scalar
                eng.dma_start(out=dst[:], in_=src)

            mh = {}
            h16 = {}
            x16 = {}
            u_open = {}
            for c in range(NCHUNK):
                mt = mp.tile([128, KI, CB], f8, name=f"mh_0_{c}", tag="mh",
                             bufs=MH_BUFS)
                nc.sync.dma_start(out=mt[:], in_=x8d[0, c])
                mh[c] = mt
                ht = wp.tile([128, 3, CB], f16, name=f"h16_i_{c}", tag="h16",
                             bufs=H_BUFS)
                nc.scalar.dma_start(out=ht[:, 0:2, :], in_=h016d[c])
                nc.scalar.dma_start(out=ht[:, 2, :], in_=h2c16d)
                h16[c] = ht
                xt = wp.tile([128, KI, CB], f16, name=f"x16_0_{c}", tag="x16",
                             bufs=4)
                nc.sync.dma_start(out=xt[:], in_=x16d[0, c])
                x16[c] = xt

            def open_u(t, c):
                # pre-open ps_u m0 (bank A) and m2 (bank B) with their
                # x-side matmuls -- at most ONE open group per PSUM bank.
                # m1 runs as a complete group in phase A after m0 closes.
                att_u = pp.tile([128, 4, CB], f32, name=f"psu_{t}_{c}",
                                tag="att", bufs=2)
                for m in (0, 2):
                    nc.tensor.matmul(att_u[:, m, :], wu8[:, 0:2, ms(m)],
                                     mh[c][:, 0:2, :], start=True,
                                     stop=False, perf_mode=DR)
                    nc.tensor.matmul(att_u[:, m, :], wu8[:, 2, ms(m)],
                                     mh[c][:, 2, :], start=False,
                                     stop=False)
                u_open[c] = att_u

            for c in range(NCHUNK):
                open_u(0, c)

            for t in range(t_steps):
                mh_nxt = {}
                if t + 1 < t_steps:
                    for c in range(NCHUNK):
                        mt = mp.tile([128, KI, CB], f8, name=f"mh_{t + 1}_{c}",
                                     tag="mh", bufs=MH_BUFS)
                        nc.sync.dma_start(out=mt[:], in_=x8d[t + 1, c])
                        mh_nxt[c] = mt
                        xt = wp.tile([128, KI, CB], f16,
                                     name=f"x16_{t + 1}_{c}", tag="x16",
                                     bufs=4)
                        nc.sync.dma_start(out=xt[:], in_=x16d[t + 1, c])
                        x16[NCHUNK + c] = xt

                st = [{} for _ in range(NCHUNK)]

                # t1h (h-only, fp16): PE filler during ACT stages
                def emit_t1h(c):
                    t1h = pp.tile([128, 2, CB], f32, name=f"t1h_{t}_{c}",
                                  tag="gp", bufs=4)
                    for mb in range(2):
                        for k in range(3):
                            nc.tensor.matmul(
                                t1h[:, mb, :], wt16[:, k, ms(mb)],
                                h16[c][:, k, :], start=(k == 0),
                                stop=(k == 2))
                    st[c]["t1h"] = t1h

                # ---- phase A: ps_u x-side then h-side, tanh ----
                for c in range(NCHUNK):
                    if c == 1:
                        emit_t1h(0)
                    att_u = u_open[c]
                    # close m0, then run m1 complete, then close m2
                    for k in range(2):
                        nc.tensor.matmul(att_u[:, 0, :], wu16[:, k, ms(0)],
                                         h16[c][:, k, :], start=False,
                                         stop=(k == 1))
                    nc.tensor.matmul(att_u[:, 1, :], wu8[:, 0:2, ms(1)],
                                     mh[c][:, 0:2, :], start=True,
                                     stop=False, perf_mode=DR)
                    nc.tensor.matmul(att_u[:, 1, :], wu8[:, 2, ms(1)],
                                     mh[c][:, 2, :], start=False, stop=False)
                    for k in range(2):
                        nc.tensor.matmul(att_u[:, 1, :], wu16[:, k, ms(1)],
                                         h16[c][:, k, :], start=False,
                                         stop=(k == 1))
                    for k in range(2):
                        nc.tensor.matmul(att_u[:, 2, :], wu16[:, k, ms(2)],
                                         h16[c][:, k, :], start=False,
                                         stop=(k == 1))
                    st[c]["att_u"] = att_u

                for c in range(NCHUNK):
                    u8 = wp.tile([128, 4, CB], f8, name=f"u8_{t}_{c}",
                                 tag="u8", bufs=U_BUFS)
                    if t * NCHUNK + c < U_BUFS:
                        nc.scalar.dma_start(out=u8[:, 3, :], in_=uz8d)
                    nc.scalar.activation(u8[:, 0:2, :],
                                         st[c]["att_u"][:, 0:2, :], AF.Tanh)
                    nc.scalar.activation(u8[:, 2, :],
                                         st[c]["att_u"][:, 2, :], AF.Tanh)
                    st[c]["u8"] = u8

                # ---- phase B: ps_v = attn3.u (+bv), exp (split) ----
                for c in range(NCHUNK):
                    att_v = pp.tile([128, 4, CB], f32, name=f"psv_{t}_{c}",
                                    tag="att", bufs=2)
                    u8 = st[c]["u8"]
                    for m in range(3):
                        for p in range(2):
                            nc.tensor.matmul(
                                att_v[:, m, :], wv8[:, p, :, ms(m)],
                                u8[:, 2 * p:2 * p + 2, :],
                                start=(p == 0), stop=(p == 1), perf_mode=DR)
                    ev = wp.tile([128, 3, CB], f16, name=f"ev_{t}_{c}",
                                 tag="ev", bufs=3)
                    nc.scalar.activation(ev[:, 0:2, :], att_v[:, 0:2, :],
                                         AF.Exp)
                    nc.scalar.activation(ev[:, 2, :], att_v[:, 2, :], AF.Exp)
                    st[c]["ev"] = ev
                    st[c]["att_v"] = att_v

                # ---- phase C: den, xev, rinv, wx ----
                for c in range(NCHUNK):
                    ev = st[c]["ev"]
                    att_v = st[c]["att_v"]
                    for k in range(3):
                        nc.tensor.matmul(att_v[:, 3, :], ones16[:],
                                         ev[:, k, :], start=(k == 0),
                                         stop=(k == 2))
                    xc = x16[c]
                    xev = wp.tile([128, 3, CB], f16, name=f"xev_{t}_{c}",
                                  tag="xev", bufs=2)
                    nc.vector.tensor_tensor(out=xev[:, 0:2, :],
                                            in0=xc[:, 0:2, :],
                                            in1=ev[:, 0:2, :], op=OP.mult)
                    nc.gpsimd.tensor_tensor(out=xev[0:64, 2, :],
                                            in0=xc[0:64, 2, :],
                                            in1=ev[0:64, 2, :], op=OP.mult)
                    rinv = wp.tile([128, CB], f32, name=f"rinv_{t}_{c}",
                                   tag="rinv", bufs=2)
                    nc.vector.reciprocal_approx_fast(rinv[:], att_v[:, 3, :])
                    _r = rinv[:]
                    rrep = bass.AP(tensor=_r.tensor, offset=_r.offset,
                                   ap=[_r.ap[0], [0, 2], _r.ap[1]])
                    nc.vector.tensor_tensor(out=mh[c][:, 0:2, :],
                                            in0=xev[:, 0:2, :], in1=rrep,
                                            op=OP.mult)
                    nc.vector.tensor_tensor(out=mh[c][0:64, 2, :],
                                            in0=xev[0:64, 2, :],
                                            in1=rinv[0:64, :], op=OP.mult)
                    # pre-open next step's ps_u m0/m2 groups (x-only)
                    if t + 1 < t_steps:
                        sv_mh = mh[c]
                        mh[c] = mh_nxt[c]
                        open_u(t + 1, c)
                        mh[c] = sv_mh

                # ---- phase D: gates + GRU tail ----
                for c in range(NCHUNK):
                    hprev = h16[c]
                    if c == 1:
                        emit_t1h(1)
                    t1h = st[c]["t1h"]
                    rr = pp.tile([128, 2, CB], f32, name=f"rps_{t}_{c}",
                                 tag="gp", bufs=4)
                    inp = pp.tile([128, 2, CB], f32, name=f"inps_{t}_{c}",
                                  tag="gp", bufs=4)
                    zz_ps = pp.tile([128, 2, CB], f32, name=f"zps_{t}_{c}",
                                    tag="gp", bufs=4)

                    def gate_block(pstile, mb, wid):
                        nc.tensor.matmul(pstile[:, mb, :],
                                         wrz8[:, 0:2, ms(wid)],
                                         mh[c][:, 0:2, :], start=True,
                                         stop=False, perf_mode=DR)
                        nc.tensor.matmul(pstile[:, mb, :],
                                         wrz8[:, 2, ms(wid)],
                                         mh[c][:, 2, :], start=False,
                                         stop=False)
                        for k in range(2):
                            nc.tensor.matmul(pstile[:, mb, :],
                                             wrz16[:, k, ms(wid)],
                                             hprev[:, k, :], start=False,
                                             stop=(k == 1))

                    for mb in range(2):
                        gate_block(rr, mb, mb)
                    g01 = wp.tile([128, 2, CB], f16, name=f"g01_{t}_{c}",
                                  tag="g01", bufs=2)
                    nc.scalar.activation(g01[:], rr[:], AF.Tanh,
                                         scale=0.5 / S)
                    for mb in range(2):
                        nc.tensor.matmul(inp[:, mb, :], wp8[:, 0:2, ms(mb)],
                                         mh[c][:, 0:2, :], start=True,
                                         stop=False, perf_mode=DR)
                        nc.tensor.matmul(inp[:, mb, :], wp8[:, 2, ms(mb)],
                                         mh[c][:, 2, :], start=False,
                                         stop=False)
                        nc.tensor.matmul(inp[:, mb, :], wbin16[:, ms(mb)],
                                         hprev[:, 2, :], start=False,
                                         stop=True)
                    for mb in range(2):
                        gate_block(zz_ps, mb, 2 + mb)
                    g23 = wp.tile([128, 2, CB], f16, name=f"g23_{t}_{c}",
                                  tag="g23", bufs=2)
                    nc.scalar.activation(g23[:], zz_ps[:], AF.Tanh,
                                         scale=0.5 / S)

                    rt = wp.tile([128, 2, CB], f16, name=f"rt_{t}_{c}",
                                 tag="rt", bufs=2)
                    nc.vector.scalar_tensor_tensor(
                        rt[:], g01[:], 1.0, t1h[:], OP.add, OP.mult)
                    s2 = wp.tile([128, 2, CB], f16, name=f"s2_{t}_{c}",
                                 tag="s2", bufs=2)
                    nc.vector.tensor_tensor(out=s2[:], in0=inp[:],
                                            in1=rt[:], op=OP.add)
                    n16 = wp.tile([128, 2, CB], f16, name=f"n_{t}_{c}",
                                  tag="n", bufs=2)
                    nc.scalar.activation(n16[:], s2[:], AF.Tanh,
                                         scale=1.0 / S)
                    zz = wp.tile([128, 2, CB], f16, name=f"zz_{t}_{c}",
                                 tag="zz", bufs=2)
                    nc.gpsimd.tensor_scalar(out=zz[:], in0=g23[:],
                                            scalar1=0.5, scalar2=0.5,
                                            op0=OP.mult, op1=OP.add)
                    w1z = wp.tile([128, 2, CB], f16, name=f"w1z_{t}_{c}",
                                  tag="w1z", bufs=2)
                    nc.vector.tensor_scalar(out=w1z[:], in0=g23[:],
                                            scalar1=-0.5, scalar2=0.5,
                                            op0=OP.mult, op1=OP.add)
                    bzh = wp.tile([128, 2, CB], f16, name=f"bzh_{t}_{c}",
                                  tag="bzh", bufs=2)
                    nc.gpsimd.tensor_tensor(out=bzh[:], in0=zz[:],
                                            in1=hprev[:, 0:2, :], op=OP.mult)
                    a4 = wp.tile([128, 2, CB], f16, name=f"a4_{t}_{c}",
                                 tag="a4", bufs=2)
                    nc.vector.tensor_tensor(out=a4[:], in0=w1z[:],
                                            in1=n16[:], op=OP.mult)
                    hnew = wp.tile([128, 3, CB], f16, name=f"h16_{t}_{c}",
                                   tag="h16", bufs=H_BUFS)
                    halloc = NCHUNK + t * NCHUNK + c
                    if halloc < H_BUFS:
                        nc.scalar.dma_start(out=hnew[:, 2, :], in_=h2c16d)
                    nc.vector.tensor_tensor(out=hnew[:, 0:2, :], in0=a4[:],
                                            in1=bzh[:], op=OP.add)
                    h16[c] = hnew
                    nc.sync.dma_start(out=outd[t, c], in_=hnew[:, 0:2, :])

                for c in range(NCHUNK):
                    if t + 1 < t_steps:
                        mh[c] = mh_nxt[c]
                        x16[c] = x16[NCHUNK + c]

    nc.compile()
    return nc


# ---------------- host-side data prep ----------------

def _prep_core_inputs(x, h0, attn1_W, attn1_b, attn2_W, attn2_b, attn3_W,
                      attn3_b, W_ih, b_ih, W_hh, b_hh, t_steps=T):
    import ml_dtypes
    f4 = np.float32
    f16n = np.float16
    f8n = ml_dtypes.float8_e4m3

    x = np.asarray(x, f4)
    h0 = np.asarray(h0, f4)

    A1T = np.zeros((H, IP), f4)
    A1T[:, :I] = np.asarray(attn1_W, f4).T
    A2T = np.zeros((IP, IP), f4)
    A2T[:I, :I] = np.asarray(attn2_W, f4).T
    A3T = np.zeros((IP, IP), f4)
    A3T[:I, :I] = np.asarray(attn3_W, f4).T
    WihT = np.zeros((IP, 3 * H), f4)
    WihT[:I, :] = np.asarray(W_ih, f4).T
    WhhT = np.asarray(W_hh, f4).T                      # [256, 768]
    bu = np.zeros(IP, f4)
    bu[:I] = np.asarray(attn1_b, f4) + np.asarray(attn2_b, f4)
    bv = np.full(IP, -448.0, f4)
    bv[:I] = np.asarray(attn3_b, f4)
    brz = (np.asarray(b_ih, f4) + np.asarray(b_hh, f4))[:2 * H]
    bhn = np.asarray(b_hh, f4)[2 * H:]
    bin_ = np.asarray(b_ih, f4)[2 * H:]

    # wu8: [128, 3, IP]: slots (x0, x1) for the DR pair, x2 plain (+bu row)
    wu = np.zeros((3, 128, IP), f4)
    wu[0] = A2T[0:128]
    wu[1] = A2T[128:256]
    wu[2] = A2T[256:384]
    wu[2, 64, :] = bu
    wu8 = np.ascontiguousarray(wu.transpose(1, 0, 2)).astype(f8n)

    # wu16: h-side of ps_u (attn1)
    wuh = np.stack([A1T[0:128], A1T[128:256]])
    wu16 = np.ascontiguousarray(wuh.transpose(1, 0, 2)).astype(f16n)

    # wv8: DR pairs (u0,u1),(u2,uz); uz row0 carries bv
    wv = np.zeros((2, 2, 128, IP), f4)
    wv[0, 0] = A3T[0:128]
    wv[0, 1] = A3T[128:256]
    wv[1, 0] = A3T[256:384]
    wv[1, 1, 0, :] = bv
    wv8 = np.ascontiguousarray(wv.transpose(2, 0, 1, 3)).astype(f8n)

    # wrz8: x-side, slots (x0,x1) DR + x2 (+S*brz row); wrz16: h-side *S
    wrz = np.zeros((3, 128, 512), f4)
    wrz[0] = WihT[0:128, :512]
    wrz[1] = WihT[128:256, :512]
    wrz[2] = WihT[256:384, :512]
    wrz[2, 64, :] = S * brz
    wrz8 = np.ascontiguousarray(wrz.transpose(1, 0, 2)).astype(f8n)
    wrzh = np.stack([S * WhhT[0:128, :512], S * WhhT[128:256, :512]])
    wrz16 = np.ascontiguousarray(wrzh.transpose(1, 0, 2)).astype(f16n)

    # wp8: i_n x-side (no bias row); wbin16: fp16 ones-row bias matmul
    wpp = np.zeros((3, 128, 256), f4)
    wpp[0] = WihT[0:128, 512:]
    wpp[1] = WihT[128:256, 512:]
    wpp[2] = WihT[256:384, 512:]
    wp8 = np.ascontiguousarray(wpp.transpose(1, 0, 2)).astype(f8n)
    wbin = np.zeros((128, 256), f4)
    wbin[0, :] = S * bin_
    wbin16 = wbin.astype(f16n)

    # wt16: t1h = S*0.5*(Whh_n.h + b_hh_n); k2 = fp16 ones-row bias
    wt = np.zeros((3, 128, 256), f4)
    wt[0] = 0.5 * S * WhhT[0:128, 512:]
    wt[1] = 0.5 * S * WhhT[128:256, 512:]
    wt[2, 0, :] = 0.5 * S * bhn
    wt16 = np.ascontiguousarray(wt.transpose(1, 0, 2)).astype(f16n)

    ones16 = np.full((128, 128), 1.0 / S, f16n)
    uz8 = np.zeros((128, CB), f8n)
    uz8[0, :] = 1.0
    h2c16 = np.zeros((128, CB), f16n)
    h2c16[0, :] = 1.0

    # x: pad to IP, plant the ones-row at feature 320 (slot2 row64)
    xp = np.zeros((B, t_steps, IP), f4)
    xp[:, :, :I] = x[:, :t_steps, :]
    xp[:, :, I] = 1.0
    xr = xp.reshape(NCORES, NCHUNK, CB, t_steps, KI, 128)
    xr = xr.transpose(0, 3, 1, 5, 4, 2)
    x8 = np.ascontiguousarray(xr).astype(f8n)
    x16 = np.ascontiguousarray(xr).astype(f16n)

    h0r = h0.reshape(NCORES, NCHUNK, CB, KH, 128).transpose(0, 1, 4, 3, 2)
    h016 = np.ascontiguousarray(h0r).astype(f16n)

    shared = dict(wu8=wu8, wv8=wv8, wrz8=wrz8, wp8=wp8, wu16=wu16,
                  wrz16=wrz16, wt16=wt16, wbin16=wbin16, ones16=ones16,
                  uz8=uz8, h2c16=h2c16)
    in_maps = []
    for c in range(NCORES):
        m = dict(shared)
        m["x8"] = x8[c]
        m["x16"] = x16[c]
        m["h016"] = h016[c]
        in_maps.append(m)
    return in_maps


def _gather(results, t_steps=T):
    outs = []
    for c in range(NCORES):
        o = np.asarray(results[c]["outT"], np.float32)
        o = o.transpose(1, 4, 0, 3, 2).reshape(BS, t_steps, H)
        outs.append(o)
    return np.ascontiguousarray(np.concatenate(outs, axis=0))


def _get_nc(t_steps=T):
    key = ("nc", t_steps)
    if key not in _STATE:
        _STATE[key] = _build(t_steps)
    return _STATE[key]


def run(inputs, trace=False, t_steps=T):
    from concourse.bass_utils import run_bass_kernel_spmd
    nc = _get_nc(t_steps)
    in_maps = _prep_core_inputs(t_steps=t_steps, **inputs)
    res = run_bass_kernel_spmd(nc, in_maps, list(range(NCORES)), trace=trace)
    return _gather(res.results, t_steps), res


def kernel(**inputs):
    out, _ = run(inputs, trace=False)
    return out


# revision 20
# speedup vs baseline: 1.0382x; 1.0382x over previous
"""Trainium2 Bass kernel for nn_Encoder_81595788689580.

Attention-gated GRU encoder: per time step
    w1 = h @ attn1_W.T + attn1_b
    w2 = x_t @ attn2_W.T + attn2_b
    v  = tanh(w1 + w2) @ attn3_W.T + attn3_b
    alpha = softmax(v, axis=feature)
    wx = x_t * alpha
    GRU cell (r, z, n) -> h_new
Output: [B, T, H] hidden states.

Strategy (8 NeuronCores, data-parallel over batch; 512 rows/core run as
2 pipelined chunks of 256):
  - transposed layout: features on partitions, batch on the free dim;
    weights-stationary matmuls, biases ride inside the matmuls as
    ones-rows planted in the zero-padding (x feature 320 = 1.0, and an
    fp16 ones-row slot in the h state tile).
  - x-side contractions in fp8(e4m3): the (x0,x1) 256-row pair runs as
    one DoubleRow matmul (2x), x2 as a plain fp8 matmul carrying the
    bias row; h-side contractions stay fp16 (exact recurrent path,
    keeps PE busy enough to hold the 2.4GHz p-state, and avoids an
    h->fp8 cast on the critical path).
  - wx is stored *S (S=16) in fp8 (normal range), descaled by the ACT
    `scale` at each gate activation; the r/z h-side weights carry S,
    the n-path h-side weights carry 0.5*S (folding the sigmoid scale).
  - precision: n-gate path fp16 end-to-end; sim rel err ~2e-3.
  - chain shortening: h-only matmuls (t1h, bias rows) fill the PE
    while ACT runs; the gate ACT is split r-first/z-second so the
    n-gate chain starts after only the r blocks; the z-path products
    (zz, w1z on DVE, bzh on gpsimd) run off the critical chain.
  - softmax denominator: fp16 ones-matmul (value 1/S) so reciprocal
    yields the S-scaled normalizer directly (read as f32 by the DVE
    alpha multiply; no cast hop).
"""

import numpy as np

B, T, I, H = 4096, 24, 320, 256
NCORES = 8
BS = B // NCORES          # 512 rows per core
NCHUNK = 2
CB = BS // NCHUNK         # 256 batch columns per chunk
IP = 384                  # I padded to 3*128
KI = IP // 128            # 3 feature blocks
KH = H // 128             # 2 hidden blocks
S = 16.0                  # wx / gate-psum scale (fp8 range)

_STATE = {}


def _build(t_steps=T):
    import concourse.bass as bass
    import concourse.tile as tile
    from concourse import bacc, mybir

    f32 = mybir.dt.float32
    f16 = mybir.dt.float16
    f8 = mybir.dt.float8e4
    AF = mybir.ActivationFunctionType
    OP = mybir.AluOpType
    DR = mybir.MatmulPerfMode.DoubleRow

    nc = bacc.Bacc("TRN2", target_bir_lowering=False, debug=False,
                   num_devices=NCORES)

    x8d = nc.dram_tensor("x8", [t_steps, NCHUNK, 128, KI, CB], f8,
                         kind="ExternalInput").ap()
    x16d = nc.dram_tensor("x16", [t_steps, NCHUNK, 128, KI, CB], f16,
                          kind="ExternalInput").ap()
    h016d = nc.dram_tensor("h016", [NCHUNK, 128, KH, CB], f16,
                           kind="ExternalInput").ap()
    wu8d = nc.dram_tensor("wu8", [128, 3, IP], f8,
                          kind="ExternalInput").ap()
    wv8d = nc.dram_tensor("wv8", [128, 2, 2, IP], f8,
                          kind="ExternalInput").ap()
    wrz8d = nc.dram_tensor("wrz8", [128, 3, 512], f8,
                           kind="ExternalInput").ap()
    wp8d = nc.dram_tensor("wp8", [128, 3, 256], f8,
                          kind="ExternalInput").ap()
    wu16d = nc.dram_tensor("wu16", [128, 2, IP], f16,
                           kind="ExternalInput").ap()
    wrz16d = nc.dram_tensor("wrz16", [128, 2, 512], f16,
                            kind="ExternalInput").ap()
    wt16d = nc.dram_tensor("wt16", [128, 3, 256], f16,
                           kind="ExternalInput").ap()
    wbin16d = nc.dram_tensor("wbin16", [128, 256], f16,
                             kind="ExternalInput").ap()
    ones16d = nc.dram_tensor("ones16", [128, 128], f16,
                             kind="ExternalInput").ap()
    uz8d = nc.dram_tensor("uz8", [128, CB], f8, kind="ExternalInput").ap()
    h2c16d = nc.dram_tensor("h2c16", [128, CB], f16,
                            kind="ExternalInput").ap()
    outd = nc.dram_tensor("outT", [t_steps, NCHUNK, 128, KH, CB], f16,
                          kind="ExternalOutput").ap()

    MH_BUFS = 4
    U_BUFS = 3
    H_BUFS = 4

    def ms(m):
        return slice(m * 128, (m + 1) * 128)

    with tile.TileContext(nc) as tc:
        with tc.tile_pool(name="const", bufs=1) as cp, \
             tc.tile_pool(name="mhp", bufs=1) as mp, \
             tc.tile_pool(name="wk", bufs=1) as wp, \
             tc.tile_pool(name="ps", bufs=1, space="PSUM") as pp:

            wu8 = cp.tile([128, 3, IP], f8)
            wv8 = cp.tile([128, 2, 2, IP], f8)
            wrz8 = cp.tile([128, 3, 512], f8)
            wp8 = cp.tile([128, 3, 256], f8)
            wu16 = cp.tile([128, 2, IP], f16)
            wrz16 = cp.tile([128, 2, 512], f16)
            wt16 = cp.tile([128, 3, 256], f16)
            wbin16 = cp.tile([128, 256], f16)
            ones16 = cp.tile([128, 128], f16)
            for i, (dst, src) in enumerate([
                    (wu8, wu8d), (wv8, wv8d), (wrz8, wrz8d), (wp8, wp8d),
                    (wu16, wu16d), (wrz16, wrz16d), (wt16, wt16d),
                    (wbin16, wbin16d), (ones16, ones16d)]):
                eng = nc.sync if i % 2 == 0 else nc.scalar
                eng.dma_start(out=dst[:], in_=src)

            mh = {}
            h16 = {}
            x16 = {}
            u_open = {}
            for c in range(NCHUNK):
                mt = mp.tile([128, KI, CB], f8, name=f"mh_0_{c}", tag="mh",
                             bufs=MH_BUFS)
                nc.sync.dma_start(out=mt[:], in_=x8d[0, c])
                mh[c] = mt
                ht = wp.tile([128, 3, CB], f16, name=f"h16_i_{c}", tag="h16",
                             bufs=H_BUFS)
                nc.scalar.dma_start(out=ht[:, 0:2, :], in_=h016d[c])
                nc.scalar.dma_start(out=ht[:, 2, :], in_=h2c16d)
                h16[c] = ht
                xt = wp.tile([128, KI, CB], f16, name=f"x16_0_{c}", tag="x16",
                             bufs=4)
                nc.sync.dma_start(out=xt[:], in_=x16d[0, c])
                x16[c] = xt

            def open_u(t, c):
                # pre-open ps_u m0 (bank A) and m2 (bank B) with their
                # x-side matmuls -- at most ONE open group per PSUM bank.
                # m1 runs as a complete group in phase A after m0 closes.
                att_u = pp.tile([128, 4, CB], f32, name=f"psu_{t}_{c}",
                                tag="att", bufs=2)
                for m in (0, 2):
                    nc.tensor.matmul(att_u[:, m, :], wu8[:, 0:2, ms(m)],
                                     mh[c][:, 0:2, :], start=True,
                                     stop=False, perf_mode=DR)
                    nc.tensor.matmul(att_u[:, m, :], wu8[:, 2, ms(m)],
                                     mh[c][:, 2, :], start=False,
                                     stop=False)
                u_open[c] = att_u

            for c in range(NCHUNK):
                open_u(0, c)

            for t in range(t_steps):
                mh_nxt = {}
                if t + 1 < t_steps:
                    for c in range(NCHUNK):
                        mt = mp.tile([128, KI, CB], f8, name=f"mh_{t + 1}_{c}",
                                     tag="mh", bufs=MH_BUFS)
                        nc.sync.dma_start(out=mt[:], in_=x8d[t + 1, c])
                        mh_nxt[c] = mt
                        xt = wp.tile([128, KI, CB], f16,
                                     name=f"x16_{t + 1}_{c}", tag="x16",
                                     bufs=4)
                        nc.sync.dma_start(out=xt[:], in_=x16d[t + 1, c])
                        x16[NCHUNK + c] = xt

                st = [{} for _ in range(NCHUNK)]

                # t1h (h-only, fp16): PE filler during ACT stages
                def emit_t1h(c):
                    t1h = pp.tile([128, 2, CB], f32, name=f"t1h_{t}_{c}",
                                  tag="gp", bufs=4)
                    for mb in range(2):
                        for k in range(3):
                            nc.tensor.matmul(
                                t1h[:, mb, :], wt16[:, k, ms(mb)],
                                h16[c][:, k, :], start=(k == 0),
                                stop=(k == 2))
                    st[c]["t1h"] = t1h

                # ---- phase A: ps_u x-side then h-side, tanh ----
                for c in range(NCHUNK):
                    if c == 1:
                        emit_t1h(0)
                    att_u = u_open[c]
                    # close m0, then run m1 complete, then close m2
                    for k in range(2):
                        nc.tensor.matmul(att_u[:, 0, :], wu16[:, k, ms(0)],
                                         h16[c][:, k, :], start=False,
                                         stop=(k == 1))
                    nc.tensor.matmul(att_u[:, 1, :], wu8[:, 0:2, ms(1)],
                                     mh[c][:, 0:2, :], start=True,
                                     stop=False, perf_mode=DR)
                    nc.tensor.matmul(att_u[:, 1, :], wu8[:, 2, ms(1)],
                                     mh[c][:, 2, :], start=False, stop=False)
                    for k in range(2):
                        nc.tensor.matmul(att_u[:, 1, :], wu16[:, k, ms(1)],
                                         h16[c][:, k, :], start=False,
                                         stop=(k == 1))
                    for k in range(2):
                        nc.tensor.matmul(att_u[:, 2, :], wu16[:, k, ms(2)],
                                         h16[c][:, k, :], start=False,
                                         stop=(k == 1))
                    st[c]["att_u"] = att_u

                for c in range(NCHUNK):
                    u8 = wp.tile([128, 4, CB], f8, name=f"u8_{t}_{c}",
                                 tag="u8", bufs=U_BUFS)
                    if t * NCHUNK + c < U_BUFS:
                        nc.scalar.dma_start(out=u8[:, 3, :], in_=uz8d)
                    nc.scalar.activation(u8[:, 0:2, :],
                                         st[c]["att_u"][:, 0:2, :], AF.Tanh)
                    nc.scalar.activation(u8[:, 2, :],
                                         st[c]["att_u"][:, 2, :], AF.Tanh)
                    st[c]["u8"] = u8

                # ---- phase B: ps_v = attn3.u (+bv), exp (split) ----
                for c in range(NCHUNK):
                    att_v = pp.tile([128, 4, CB], f32, name=f"psv_{t}_{c}",
                                    tag="att", bufs=2)
                    u8 = st[c]["u8"]
                    for m in range(3):
                        for p in range(2):
                            nc.tensor.matmul(
                                att_v[:, m, :], wv8[:, p, :, ms(m)],
                                u8[:, 2 * p:2 * p + 2, :],
                                start=(p == 0), stop=(p == 1), perf_mode=DR)
                    ev = wp.tile([128, 3, CB], f16, name=f"ev_{t}_{c}",
                                 tag="ev", bufs=3)
                    nc.scalar.activation(ev[:, 0:2, :], att_v[:, 0:2, :],
                                         AF.Exp)
                    nc.scalar.activation(ev[:, 2, :], att_v[:, 2, :], AF.Exp)
                    st[c]["ev"] = ev
                    st[c]["att_v"] = att_v

                # ---- phase C: den, xev, rinv, wx ----
                for c in range(NCHUNK):
                    ev = st[c]["ev"]
                    att_v = st[c]["att_v"]
                    for k in range(3):
                        nc.tensor.matmul(att_v[:, 3, :], ones16[:],
                                         ev[:, k, :], start=(k == 0),
                                         stop=(k == 2))
                    xc = x16[c]
                    xev = wp.tile([128, 3, CB], f16, name=f"xev_{t}_{c}",
                                  tag="xev", bufs=2)
                    nc.vector.tensor_tensor(out=xev[:, 0:2, :],
                                            in0=xc[:, 0:2, :],
                                            in1=ev[:, 0:2, :], op=OP.mult)
                    nc.gpsimd.tensor_tensor(out=xev[0:64, 2, :],
                                            in0=xc[0:64, 2, :],
                                            in1=ev[0:64, 2, :], op=OP.mult)
                    rinv = wp.tile([128, CB], f32, name=f"rinv_{t}_{c}",
                                   tag="rinv", bufs=2)
                    nc.vector.reciprocal_approx_fast(rinv[:], att_v[:, 3, :])
                    _r = rinv[:]
                    rrep = bass.AP(tensor=_r.tensor, offset=_r.offset,
                                   ap=[_r.ap[0], [0, 2], _r.ap[1]])
                    nc.vector.tensor_tensor(out=mh[c][:, 0:2, :],
                                            in0=xev[:, 0:2, :], in1=rrep,
                                            op=OP.mult)
                    nc.vector.tensor_tensor(out=mh[c][0:64, 2, :],
                                            in0=xev[0:64, 2, :],
                                            in1=rinv[0:64, :], op=OP.mult)
                    # pre-open next step's ps_u m0/m2 groups (x-only)
                    if t + 1 < t_steps:
                        sv_mh = mh[c]
                        mh[c] = mh_nxt[c]
                        open_u(t + 1, c)
                        mh[c] = sv_mh

                # ---- phase D: gates + GRU tail ----
                for c in range(NCHUNK):
                    hprev = h16[c]
                    if c == 1:
                        emit_t1h(1)
                    t1h = st[c]["t1h"]
                    rr = pp.tile([128, 2, CB], f32, name=f"rps_{t}_{c}",
                                 tag="gp", bufs=4)
                    inp = pp.tile([128, 2, CB], f32, name=f"inps_{t}_{c}",
                                  tag="gp", bufs=4)
                    zz_ps = pp.tile([128, 2, CB], f32, name=f"zps_{t}_{c}",
                                    tag="gp", bufs=4)

                    def gate_block(pstile, mb, wid):
                        nc.tensor.matmul(pstile[:, mb, :],
                                         wrz8[:, 0:2, ms(wid)],
                                         mh[c][:, 0:2, :], start=True,
                                         stop=False, perf_mode=DR)
                        nc.tensor.matmul(pstile[:, mb, :],
                                         wrz8[:, 2, ms(wid)],
                                         mh[c][:, 2, :], start=False,
                                         stop=False)
                        for k in range(2):
                            nc.tensor.matmul(pstile[:, mb, :],
                                             wrz16[:, k, ms(wid)],
                                             hprev[:, k, :], start=False,
                                             stop=(k == 1))

                    for mb in range(2):
                        gate_block(rr, mb, mb)
                    g01 = wp.tile([128, 2, CB], f16, name=f"g01_{t}_{c}",
                                  tag="g01", bufs=2)
                    nc.scalar.activation(g01[:], rr[:], AF.Tanh,
                                         scale=0.5 / S)
                    for mb in range(2):
                        nc.tensor.matmul(inp[:, mb, :], wp8[:, 0:2, ms(mb)],
                                         mh[c][:, 0:2, :], start=True,
                                         stop=False, perf_mode=DR)
                        nc.tensor.matmul(inp[:, mb, :], wp8[:, 2, ms(mb)],
                                         mh[c][:, 2, :], start=False,
                                         stop=False)
                        nc.tensor.matmul(inp[:, mb, :], wbin16[:, ms(mb)],
                                         hprev[:, 2, :], start=False,
                                         stop=True)
                    for mb in range(2):
                        gate_block(zz_ps, mb, 2 + mb)
                    g23 = wp.tile([128, 2, CB], f16, name=f"g23_{t}_{c}",
                                  tag="g23", bufs=2)
                    nc.scalar.activation(g23[:], zz_ps[:], AF.Tanh,
                                         scale=0.5 / S)

                    rt = wp.tile([128, 2, CB], f16, name=f"rt_{t}_{c}",
                                 tag="rt", bufs=2)
                    nc.vector.scalar_tensor_tensor(
                        rt[:], g01[:], 1.0, t1h[:], OP.add, OP.mult)
                    s2 = wp.tile([128, 2, CB], f16, name=f"s2_{t}_{c}",
                                 tag="s2", bufs=2)
                    nc.vector.tensor_tensor(out=s2[:], in0=inp[:],
                                            in1=rt[:], op=OP.add)
                    n16 = wp.tile([128, 2, CB], f16, name=f"n_{t}_{c}",
                                  tag="n", bufs=2)
                    nc.scalar.activation(n16[:], s2[:], AF.Tanh,
                                         scale=1.0 / S)
                    zz = wp.tile([128, 2, CB], f16, name=f"zz_{t}_{c}",
                                 tag="zz", bufs=2)
                    nc.vector.tensor_scalar(out=zz[:], in0=g23[:],
                                            scalar1=0.5, scalar2=0.5,
                                            op0=OP.mult, op1=OP.add)
                    w1z = wp.tile([128, 2, CB], f16, name=f"w1z_{t}_{c}",
                                  tag="w1z", bufs=2)
                    nc.vector.tensor_scalar(out=w1z[:], in0=g23[:],
                                            scalar1=-0.5, scalar2=0.5,
                                            op0=OP.mult, op1=OP.add)
                    bzh = wp.tile([128, 2, CB], f16, name=f"bzh_{t}_{c}",
                                  tag="bzh", bufs=2)
                    nc.vector.tensor_tensor(out=bzh[:], in0=zz[:],
                                            in1=hprev[:, 0:2, :], op=OP.mult)
                    a4 = wp.tile([128, 2, CB], f16, name=f"a4_{t}_{c}",
                                 tag="a4", bufs=2)
                    nc.vector.tensor_tensor(out=a4[:], in0=w1z[:],
                                            in1=n16[:], op=OP.mult)
                    hnew = wp.tile([128, 3, CB], f16, name=f"h16_{t}_{c}",
                                   tag="h16", bufs=H_BUFS)
                    halloc = NCHUNK + t * NCHUNK + c
                    if halloc < H_BUFS:
                        nc.scalar.dma_start(out=hnew[:, 2, :], in_=h2c16d)
                    nc.vector.tensor_tensor(out=hnew[:, 0:2, :], in0=a4[:],
                                            in1=bzh[:], op=OP.add)
                    h16[c] = hnew
                    nc.sync.dma_start(out=outd[t, c], in_=hnew[:, 0:2, :])

                for c in range(NCHUNK):
                    if t + 1 < t_steps:
                        mh[c] = mh_nxt[c]
                        x16[c] = x16[NCHUNK + c]

    nc.compile()
    return nc


# ---------------- host-side data prep ----------------

def _prep_core_inputs(x, h0, attn1_W, attn1_b, attn2_W, attn2_b, attn3_W,
                      attn3_b, W_ih, b_ih, W_hh, b_hh, t_steps=T):
    import ml_dtypes
    f4 = np.float32
    f16n = np.float16
    f8n = ml_dtypes.float8_e4m3

    x = np.asarray(x, f4)
    h0 = np.asarray(h0, f4)

    A1T = np.zeros((H, IP), f4)
    A1T[:, :I] = np.asarray(attn1_W, f4).T
    A2T = np.zeros((IP, IP), f4)
    A2T[:I, :I] = np.asarray(attn2_W, f4).T
    A3T = np.zeros((IP, IP), f4)
    A3T[:I, :I] = np.asarray(attn3_W, f4).T
    WihT = np.zeros((IP, 3 * H), f4)
    WihT[:I, :] = np.asarray(W_ih, f4).T
    WhhT = np.asarray(W_hh, f4).T                      # [256, 768]
    bu = np.zeros(IP, f4)
    bu[:I] = np.asarray(attn1_b, f4) + np.asarray(attn2_b, f4)
    bv = np.full(IP, -448.0, f4)
    bv[:I] = np.asarray(attn3_b, f4)
    brz = (np.asarray(b_ih, f4) + np.asarray(b_hh, f4))[:2 * H]
    bhn = np.asarray(b_hh, f4)[2 * H:]
    bin_ = np.asarray(b_ih, f4)[2 * H:]

    # wu8: [128, 3, IP]: slots (x0, x1) for the DR pair, x2 plain (+bu row)
    wu = np.zeros((3, 128, IP), f4)
    wu[0] = A2T[0:128]
    wu[1] = A2T[128:256]
    wu[2] = A2T[256:384]
    wu[2, 64, :] = bu
    wu8 = np.ascontiguousarray(wu.transpose(1, 0, 2)).astype(f8n)

    # wu16: h-side of ps_u (attn1)
    wuh = np.stack([A1T[0:128], A1T[128:256]])
    wu16 = np.ascontiguousarray(wuh.transpose(1, 0, 2)).astype(f16n)

    # wv8: DR pairs (u0,u1),(u2,uz); uz row0 carries bv
    wv = np.zeros((2, 2, 128, IP), f4)
    wv[0, 0] = A3T[0:128]
    wv[0, 1] = A3T[128:256]
    wv[1, 0] = A3T[256:384]
    wv[1, 1, 0, :] = bv
    wv8 = np.ascontiguousarray(wv.transpose(2, 0, 1, 3)).astype(f8n)

    # wrz8: x-side, slots (x0,x1) DR + x2 (+S*brz row); wrz16: h-side *S
    wrz = np.zeros((3, 128, 512), f4)
    wrz[0] = WihT[0:128, :512]
    wrz[1] = WihT[128:256, :512]
    wrz[2] = WihT[256:384, :512]
    wrz[2, 64, :] = S * brz
    wrz8 = np.ascontiguousarray(wrz.transpose(1, 0, 2)).astype(f8n)
    wrzh = np.stack([S * WhhT[0:128, :512], S * WhhT[128:256, :512]])
    wrz16 = np.ascontiguousarray(wrzh.transpose(1, 0, 2)).astype(f16n)

    # wp8: i_n x-side (no bias row); wbin16: fp16 ones-row bias matmul
    wpp = np.zeros((3, 128, 256), f4)
    wpp[0] = WihT[0:128, 512:]
    wpp[1] = WihT[128:256, 512:]
    wpp[2] = WihT[256:384, 512:]
    wp8 = np.ascontiguousarray(wpp.transpose(1, 0, 2)).astype(f8n)
    wbin = np.zeros((128, 256), f4)
    wbin[0, :] = S * bin_
    wbin16 = wbin.astype(f16n)

    # wt16: t1h = S*0.5*(Whh_n.h + b_hh_n); k2 = fp16 ones-row bias
    wt = np.zeros((3, 128, 256), f4)
    wt[0] = 0.5 * S * WhhT[0:128, 512:]
    wt[1] = 0.5 * S * WhhT[128:256, 512:]
    wt[2, 0, :] = 0.5 * S * bhn
    wt16 = np.ascontiguousarray(wt.transpose(1, 0, 2)).astype(f16n)

    ones16 = np.full((128, 128), 1.0 / S, f16n)
    uz8 = np.zeros((128, CB), f8n)
    uz8[0, :] = 1.0
    h2c16 = np.zeros((128, CB), f16n)
    h2c16[0, :] = 1.0

    # x: pad to IP, plant the ones-row at feature 320 (slot2 row64)
    xp = np.zeros((B, t_steps, IP), f4)
    xp[:, :, :I] = x[:, :t_steps, :]
    xp[:, :, I] = 1.0
    xr = xp.reshape(NCORES, NCHUNK, CB, t_steps, KI, 128)
    xr = xr.transpose(0, 3, 1, 5, 4, 2)
    x8 = np.ascontiguousarray(xr).astype(f8n)
    x16 = np.ascontiguousarray(xr).astype(f16n)

    h0r = h0.reshape(NCORES, NCHUNK, CB, KH, 128).transpose(0, 1, 4, 3, 2)
    h016 = np.ascontiguousarray(h0r).astype(f16n)

    shared = dict(wu8=wu8, wv8=wv8, wrz8=wrz8, wp8=wp8, wu16=wu16,
                  wrz16=wrz16, wt16=wt16, wbin16=wbin16, ones16=ones16,
                  uz8=uz8, h2c16=h2c16)
    in_maps = []
    for c in range(NCORES):
        m = dict(shared)
        m["x8"] = x8[c]
        m["x16"] = x16[c]
        m["h016"] = h016[c]
        in_maps.append(m)
    return in_maps


def _gather(results, t_steps=T):
    outs = []
    for c in range(NCORES):
        o = np.asarray(results[c]["outT"], np.float32)
        o = o.transpose(1, 4, 0, 3, 2).reshape(BS, t_steps, H)
        outs.append(o)
    return np.ascontiguousarray(np.concatenate(outs, axis=0))


def _get_nc(t_steps=T):
    key = ("nc", t_steps)
    if key not in _STATE:
        _STATE[key] = _build(t_steps)
    return _STATE[key]


def run(inputs, trace=False, t_steps=T):
    from concourse.bass_utils import run_bass_kernel_spmd
    nc = _get_nc(t_steps)
    in_maps = _prep_core_inputs(t_steps=t_steps, **inputs)
    res = run_bass_kernel_spmd(nc, in_maps, list(range(NCORES)), trace=trace)
    return _gather(res.results, t_steps), res


def kernel(**inputs):
    out, _ = run(inputs, trace=False)
    return out


# revision 21
# speedup vs baseline: 1.1203x; 1.0791x over previous
"""Trainium2 Bass kernel for nn_Encoder_81595788689580.

Attention-gated GRU encoder: per time step
    w1 = h @ attn1_W.T + attn1_b
    w2 = x_t @ attn2_W.T + attn2_b
    v  = tanh(w1 + w2) @ attn3_W.T + attn3_b
    alpha = softmax(v, axis=feature)
    wx = x_t * alpha
    GRU cell (r, z, n) -> h_new
Output: [B, T, H] hidden states.

Strategy (8 NeuronCores, data-parallel over batch):
  - batch 4096 -> 512 rows per core; all weights replicated.
  - everything stored TRANSPOSED on chip: features on partitions, batch on
    the free dim. Every matmul is weights-stationary with batch as the
    moving dim, biases become per-partition ACT bias vectors, and no
    transposes are ever needed on device (host pre-/post-transposes).
  - feature dim I=320 zero-padded to 384 = 3x128 partition blocks; padded
    attn3_b rows are -1e4 so exp() of pad rows is exactly 0 and the
    softmax denominator is unaffected.
  - softmax over features is a partition reduction: an all-ones stationary
    matmul broadcasts the per-column denominator into all 128 partitions
    of one PSUM tile; max-subtraction is skipped (|v| <= ~8 in practice,
    exp stays finite, softmax is shift-invariant).
  - sigmoid is computed as 0.5*tanh(x/2)+0.5 so every ACT op uses the
    exp_and_others table set -- avoids ~2.7us ACT table swaps per step.
  - matmuls in fp16 (1 PE cycle/row, fast weight loads) with fp32 PSUM
    accumulation; attn3_b carries a -2 shift for fp16 exp range. DT="f32r" switches
    to float32r matmuls (~10x lower error, ~1.7x slower weight loads).
  - the 512-row batch runs as 2 independent chunks of 256 so the two
    recurrences pipeline against each other across engines.
"""

import numpy as np

B, T, I, H = 4096, 24, 320, 256
NCORES = 8
BS = B // NCORES          # 512 rows per core
IP = 384                  # I padded to 3*128
KI = IP // 128            # 3 feature blocks
KH = H // 128             # 2 hidden blocks
G = 3 * H                 # 768 gate rows
NCHUNK = 2
CB = BS // NCHUNK         # 256 batch columns per chunk

DT = "f16"                # "f16" | "f32r"

_STATE = {}


def _np_dt(mdt):
    from concourse import mybir
    return mybir.dt.np(mdt)


def _dts():
    from concourse import mybir
    if DT == "f16":
        return mybir.dt.float16, mybir.dt.float16
    return mybir.dt.float32r, mybir.dt.float32r


def _build(t_steps=T):
    import concourse.bass as bass
    import concourse.tile as tile
    from concourse import bacc, mybir

    f32 = mybir.dt.float32
    MMD, EVD = _dts()
    AF = mybir.ActivationFunctionType
    OP = mybir.AluOpType

    nc = bacc.Bacc("TRN2", target_bir_lowering=False, debug=False,
                   num_devices=NCORES)

    xT = nc.dram_tensor("xT", [t_steps, 128, KI, BS], MMD,
                        kind="ExternalInput").ap()
    h0T = nc.dram_tensor("h0T", [128, KH, BS], MMD, kind="ExternalInput").ap()
    wat1 = nc.dram_tensor("wat1", [128, KH, IP], MMD, kind="ExternalInput").ap()
    wat2 = nc.dram_tensor("wat2", [128, KI, IP], MMD, kind="ExternalInput").ap()
    wat3 = nc.dram_tensor("wat3", [128, KI, IP], MMD, kind="ExternalInput").ap()
    wih = nc.dram_tensor("wih", [128, KI, G], MMD, kind="ExternalInput").ap()
    whh = nc.dram_tensor("whh", [128, KH, G], MMD, kind="ExternalInput").ap()
    onesw = nc.dram_tensor("onesw", [128, 128], EVD, kind="ExternalInput").ap()
    bias_u_d = nc.dram_tensor("bias_u", [128, KI], f32, kind="ExternalInput").ap()
    bias_v_d = nc.dram_tensor("bias_v", [128, KI], f32, kind="ExternalInput").ap()
    # rz bias pre-halved for the tanh-based sigmoid
    bias_rzh_d = nc.dram_tensor("bias_rzh", [128, 4], f32,
                                kind="ExternalInput").ap()
    bias_hn_d = nc.dram_tensor("bias_hn", [128, 2], f32, kind="ExternalInput").ap()
    bias_in_d = nc.dram_tensor("bias_in", [128, 2], f32, kind="ExternalInput").ap()
    outT = nc.dram_tensor("outT", [t_steps, 128, KH, BS], MMD,
                          kind="ExternalOutput").ap()

    def fv(ap):
        # readable view for DVE of matmul-dtype tiles
        if DT == "f32r":
            return ap.bitcast(f32)
        return ap

    with tile.TileContext(nc) as tc:
        with tc.tile_pool(name="const", bufs=1) as cp, \
             tc.tile_pool(name="xs", bufs=1) as xp, \
             tc.tile_pool(name="hs", bufs=1) as hp, \
             tc.tile_pool(name="wk", bufs=1) as wp, \
             tc.tile_pool(name="ps", bufs=1, space="PSUM") as pp:

            w1t = cp.tile([128, KH, IP], MMD)
            w2t = cp.tile([128, KI, IP], MMD)
            w3t = cp.tile([128, KI, IP], MMD)
            wiht = cp.tile([128, KI, G], MMD)
            whht = cp.tile([128, KH, G], MMD)
            onest = cp.tile([128, 128], EVD)
            bu = cp.tile([128, KI], f32)
            bv = cp.tile([128, KI], f32)
            brzh = cp.tile([128, 4], f32)
            bhn = cp.tile([128, 2], f32)
            bin_ = cp.tile([128, 2], f32)
            # h0 + step-0 x first (they gate the first matmuls), then
            # weights ordered by first use, alternating the two HWDGE rings
            hcur = []
            for ci in range(NCHUNK):
                hc = hp.tile([128, KH, CB], MMD, name=f"h_{ci}",
                             tag=f"h{ci}", bufs=2)
                nc.scalar.dma_start(
                    out=hc[:], in_=h0T[:, :, ci * CB:(ci + 1) * CB])
                hcur.append(hc)
            x_pre = xp.tile([128, KI, BS], MMD, name="x_pre", tag="x", bufs=4)
            nc.sync.dma_start(out=x_pre[:], in_=xT[0])
            for i, (dst, src) in enumerate([
                    (w2t, wat2), (w1t, wat1), (bu, bias_u_d),
                    (w3t, wat3), (bv, bias_v_d), (onest, onesw),
                    (whht, whh), (wiht, wih),
                    (brzh, bias_rzh_d), (bhn, bias_hn_d),
                    (bin_, bias_in_d)]):
                eng = nc.sync if i % 2 == 0 else nc.scalar
                eng.dma_start(out=dst[:], in_=src)

            def ms(m):
                return slice(m * 128, (m + 1) * 128)

            for t in range(t_steps):
                if t == 0:
                    x_t = x_pre
                else:
                    x_t = xp.tile([128, KI, BS], MMD, name=f"x_{t}",
                                  tag="x", bufs=4)
                    nc.sync.dma_start(out=x_t[:], in_=xT[t])

                st = [{} for _ in range(NCHUNK)]

                # ---- phase 1: h-gate matmuls + attention stage 1 ----
                for ci in range(NCHUNK):
                    cs = slice(ci * CB, (ci + 1) * CB)
                    h = hcur[ci]
                    ps_u = [pp.tile([128, CB], f32,
                                    name=f"psu{m}_{t}_{ci}", tag="aps",
                                    bufs=5) for m in range(KI)]
                    for m in range(KI):
                        for k in range(KI):
                            nc.tensor.matmul(
                                ps_u[m][:], w2t[:, k, ms(m)],
                                x_t[:, k, cs], start=(k == 0), stop=False)
                        for k in range(KH):
                            nc.tensor.matmul(
                                ps_u[m][:], w1t[:, k, ms(m)],
                                h[:, k, :], start=False, stop=(k == KH - 1))
                    u = wp.tile([128, KI, CB], MMD, name=f"u_{t}_{ci}",
                                tag="u", bufs=3)
                    for m in range(KI):
                        nc.scalar.activation(u[:, m, :], ps_u[m][:],
                                             AF.Tanh, bias=bu[:, m:m + 1])
                    st[ci].update(u=u)

                # ---- phase 2: v, softmax, wx ----
                for ci in range(NCHUNK):
                    cs = slice(ci * CB, (ci + 1) * CB)
                    u = st[ci]["u"]
                    ps_v = [pp.tile([128, CB], f32,
                                    name=f"psv{m}_{t}_{ci}", tag="aps",
                                    bufs=5) for m in range(KI)]
                    for m in range(KI):
                        for k in range(KI):
                            nc.tensor.matmul(
                                ps_v[m][:], w3t[:, k, ms(m)],
                                u[:, k, :], start=(k == 0), stop=(k == KI - 1))
                    ev = wp.tile([128, KI, CB], EVD, name=f"ev_{t}_{ci}",
                                 tag="ev", bufs=3)
                    for m in range(KI):
                        nc.scalar.activation(ev[:, m, :], ps_v[m][:],
                                             AF.Exp, bias=bv[:, m:m + 1])
                    ps_den = pp.tile([128, CB], f32, name=f"psden_{t}_{ci}",
                                     tag="aps", bufs=5)
                    for k in range(KI):
                        nc.tensor.matmul(ps_den[:], onest[:], ev[:, k, :],
                                         start=(k == 0), stop=(k == KI - 1))
                    rinv = wp.tile([128, CB], f32, name=f"rinv_{t}_{ci}",
                                   tag="rinv", bufs=3)
                    nc.vector.reciprocal_approx_fast(rinv[:], ps_den[:])
                    rinv16 = wp.tile([128, CB], MMD, name=f"rinv16_{t}_{ci}",
                                     tag="rinv16", bufs=3)
                    nc.vector.tensor_copy(rinv16[:], rinv[:])
                    wx = wp.tile([128, KI, CB], MMD, name=f"wx_{t}_{ci}",
                                 tag="wx", bufs=3)
                    nc.vector.tensor_mul(wx[:], fv(x_t[:, :, cs]), fv(ev[:]))
                    _r = rinv16[:]
                    nc.vector.tensor_mul(wx[:, 0, :], fv(wx[:, 0, :]), _r)
                    rrep = bass.AP(tensor=_r.tensor, offset=_r.offset,
                                   ap=[_r.ap[0], [0, KI - 1], _r.ap[1]])
                    nc.vector.tensor_mul(wx[:, 1:KI, :], fv(wx[:, 1:KI, :]),
                                         rrep)
                    st[ci].update(wx=wx)

                # ---- phase 3: gate matmuls + GRU tail ----
                for ci in range(NCHUNK):
                    cs = slice(ci * CB, (ci + 1) * CB)
                    h = hcur[ci]
                    wx = st[ci]["wx"]
                    ps_hn = pp.tile([128, 2, CB], f32, name=f"pshn_{t}_{ci}",
                                    tag="gps", bufs=3)
                    for m in range(2):
                        for k in range(KH):
                            nc.tensor.matmul(
                                ps_hn[:, m, :], whht[:, k, ms(4 + m)],
                                h[:, k, :], start=(k == 0), stop=(k == KH - 1))
                    ps_r = pp.tile([128, 2, CB], f32, name=f"psr_{t}_{ci}",
                                   tag="gps", bufs=3)
                    ps_z = pp.tile([128, 2, CB], f32, name=f"psz_{t}_{ci}",
                                   tag="gps", bufs=3)
                    # h-only whh matmuls of the m0 slices first (r and z are
                    # different banks, so both groups may be open at once):
                    # they keep the in-order PE stream fed while wx lands
                    for mm_t, base in ((ps_r, 0), (ps_z, 2)):
                        for k in range(KH):
                            nc.tensor.matmul(
                                mm_t[:, 0, :], whht[:, k, ms(base)],
                                h[:, k, :], start=(k == 0), stop=False)
                    for mm_t, base in ((ps_r, 0), (ps_z, 2)):
                        for k in range(KI):
                            nc.tensor.matmul(
                                mm_t[:, 0, :], wiht[:, k, ms(base)],
                                wx[:, k, :], start=False, stop=(k == KI - 1))
                        for k in range(KH):
                            nc.tensor.matmul(
                                mm_t[:, 1, :], whht[:, k, ms(base + 1)],
                                h[:, k, :], start=(k == 0), stop=False)
                        for k in range(KI):
                            nc.tensor.matmul(
                                mm_t[:, 1, :], wiht[:, k, ms(base + 1)],
                                wx[:, k, :], start=False, stop=(k == KI - 1))
                    ps_in = pp.tile([128, 2, CB], f32, name=f"psin_{t}_{ci}",
                                    tag="gps", bufs=3)
                    for m in range(2):
                        for k in range(KI):
                            nc.tensor.matmul(
                                ps_in[:, m, :], wiht[:, k, ms(4 + m)],
                                wx[:, k, :], start=(k == 0), stop=(k == KI - 1))

                    g = wp.tile([128, 4, CB], MMD, name=f"g_{t}_{ci}",
                                tag="g", bufs=3)
                    for m in range(4):
                        src_ps = ps_r if m < 2 else ps_z
                        nc.scalar.activation(g[:, m, :], src_ps[:, m % 2, :],
                                             AF.Tanh, bias=brzh[:, m:m + 1],
                                             scale=0.5)
                    t1h = wp.tile([128, 2, CB], MMD, name=f"t1h_{t}_{ci}",
                                  tag="t1h", bufs=3)
                    for m in range(2):
                        nc.vector.tensor_scalar(
                            out=t1h[:, m, :], in0=ps_hn[:, m, :],
                            scalar1=bhn[:, m:m + 1], scalar2=0.5,
                            op0=OP.add, op1=OP.mult)
                    # p = (i_n + b_in) + t1h is g-independent: compute it
                    # early so only two fp16 DVE ops trail the gate ACT
                    p_ = wp.tile([128, 2, CB], MMD, name=f"p_{t}_{ci}",
                                 tag="p", bufs=3)
                    for m in range(2):
                        nc.vector.scalar_tensor_tensor(
                            p_[:, m, :], ps_in[:, m, :], bin_[:, m:m + 1],
                            t1h[:, m, :], OP.add, OP.add)
                    t0h = wp.tile([128, 2, CB], MMD, name=f"t0h_{t}_{ci}",
                                  tag="t0h", bufs=3)
                    nc.vector.tensor_mul(t0h[:], t1h[:], g[:, 0:2, :])
                    s2 = wp.tile([128, 2, CB], MMD, name=f"s2_{t}_{ci}",
                                 tag="s2", bufs=3)
                    nc.vector.tensor_add(s2[:], t0h[:], p_[:])
                    n = wp.tile([128, 2, CB], MMD, name=f"n_{t}_{ci}",
                                tag="n", bufs=3)
                    nc.scalar.activation(n[:], s2[:], AF.Tanh)

                    zz = wp.tile([128, 2, CB], MMD, name=f"zz_{t}_{ci}",
                                 tag="zz", bufs=3)
                    nc.vector.tensor_scalar(
                        out=zz[:], in0=g[:, 2:4, :], scalar1=0.5, scalar2=0.5,
                        op0=OP.mult, op1=OP.add)
                    w1z = wp.tile([128, 2, CB], MMD, name=f"w1z_{t}_{ci}",
                                  tag="w1z", bufs=3)
                    nc.vector.tensor_scalar(
                        out=w1z[:], in0=g[:, 2:4, :], scalar1=-0.5,
                        scalar2=0.5, op0=OP.mult, op1=OP.add)
                    bzh = wp.tile([128, 2, CB], MMD, name=f"bzh_{t}_{ci}",
                                  tag="bzh", bufs=3)
                    nc.vector.tensor_mul(bzh[:], zz[:], fv(h[:]))
                    a4 = wp.tile([128, 2, CB], MMD, name=f"a4_{t}_{ci}",
                                 tag="a4", bufs=3)
                    nc.vector.tensor_mul(a4[:], w1z[:], n[:])
                    h_new = hp.tile([128, KH, CB], MMD, name=f"hn_{t}_{ci}",
                                    tag=f"h{ci}", bufs=2)
                    nc.vector.tensor_add(h_new[:], a4[:], bzh[:])
                    hcur[ci] = h_new

                    nc.sync.dma_start(out=outT[t][:, :, cs], in_=h_new[:])

    nc.compile()
    return nc


# ---------------- host-side data prep ----------------

def _prep_core_inputs(x, h0, attn1_W, attn1_b, attn2_W, attn2_b, attn3_W,
                      attn3_b, W_ih, b_ih, W_hh, b_hh, t_steps=T):
    f4 = np.float32
    MMD, EVD = _dts()
    mnp = _np_dt(MMD)
    enp = _np_dt(EVD)
    x = np.asarray(x, f4)
    h0 = np.asarray(h0, f4)

    A1 = np.asarray(attn1_W, f4)                       # [I, H]
    w1 = np.zeros((H, IP), f4)
    w1[:, :I] = A1.T                                   # lhsT[hh, ii]
    wat1 = np.ascontiguousarray(
        w1.reshape(KH, 128, IP).transpose(1, 0, 2)).astype(mnp)

    A2 = np.asarray(attn2_W, f4)                       # [I, I] (out, in)
    w2 = np.zeros((IP, IP), f4)
    w2[:I, :I] = A2.T                                  # lhsT[in, out]
    wat2 = np.ascontiguousarray(
        w2.reshape(KI, 128, IP).transpose(1, 0, 2)).astype(mnp)

    A3 = np.asarray(attn3_W, f4)
    w3 = np.zeros((IP, IP), f4)
    w3[:I, :I] = A3.T
    wat3 = np.ascontiguousarray(
        w3.reshape(KI, 128, IP).transpose(1, 0, 2)).astype(mnp)

    Wi = np.asarray(W_ih, f4)                          # [G, I]
    wi = np.zeros((IP, G), f4)
    wi[:I, :] = Wi.T
    wih = np.ascontiguousarray(
        wi.reshape(KI, 128, G).transpose(1, 0, 2)).astype(mnp)

    Wh = np.asarray(W_hh, f4)                          # [G, H]
    whh = np.ascontiguousarray(
        Wh.T.reshape(KH, 128, G).transpose(1, 0, 2)).astype(mnp)

    onesw = np.ones((128, 128), enp)

    bu = np.zeros(IP, f4)
    bu[:I] = np.asarray(attn1_b, f4) + np.asarray(attn2_b, f4)
    bias_u = np.ascontiguousarray(bu.reshape(KI, 128).T)
    bvv = np.full(IP, -1e4, f4)
    bvv[:I] = np.asarray(attn3_b, f4) - 2.0   # shift-invariant, fp16 range
    bias_v = np.ascontiguousarray(bvv.reshape(KI, 128).T)
    brz = (np.asarray(b_ih, f4) + np.asarray(b_hh, f4))[:2 * H] * 0.5
    bias_rzh = np.ascontiguousarray(brz.reshape(4, 128).T)
    bias_hn = np.ascontiguousarray(
        np.asarray(b_hh, f4)[2 * H:].reshape(2, 128).T)
    bias_in = np.ascontiguousarray(
        np.asarray(b_ih, f4)[2 * H:].reshape(2, 128).T)

    x16 = x[:, :t_steps, :].astype(mnp)
    xpad = np.pad(x16, ((0, 0), (0, 0), (0, IP - I)))
    # [NC, BS, T, KI, 128] -> [NC, T, 128, KI, BS]
    xr = xpad.reshape(NCORES, BS, t_steps, KI, 128).transpose(0, 2, 4, 3, 1)
    h0r = h0.astype(mnp).reshape(NCORES, BS, KH, 128).transpose(0, 3, 2, 1)

    shared = dict(wat1=wat1, wat2=wat2, wat3=wat3, wih=wih, whh=whh,
                  onesw=onesw, bias_u=bias_u, bias_v=bias_v,
                  bias_rzh=bias_rzh, bias_hn=bias_hn, bias_in=bias_in)
    in_maps = []
    for c in range(NCORES):
        m = dict(shared)
        m["xT"] = np.ascontiguousarray(xr[c])
        m["h0T"] = np.ascontiguousarray(h0r[c])
        in_maps.append(m)
    return in_maps


def _gather(results, t_steps=T):
    outs = []
    for c in range(NCORES):
        o = np.asarray(results[c]["outT"], np.float32)
        outs.append(o.transpose(3, 0, 2, 1).reshape(BS, t_steps, H))
    return np.ascontiguousarray(np.concatenate(outs, axis=0))


def _get_nc(t_steps=T):
    key = ("nc", t_steps, DT)
    if key not in _STATE:
        _STATE[key] = _build(t_steps)
    return _STATE[key]


def run(inputs, trace=False, t_steps=T):
    from concourse.bass_utils import run_bass_kernel_spmd
    nc = _get_nc(t_steps)
    in_maps = _prep_core_inputs(t_steps=t_steps, **inputs)
    res = run_bass_kernel_spmd(nc, in_maps, list(range(NCORES)), trace=trace)
    return _gather(res.results, t_steps), res


def kernel(**inputs):
    out, _ = run(inputs, trace=False)
    return out

